# revision 31
# baseline (speedup 1.0000x reference)
"""2-layer GCN (GCNConv + LayerNorm + ReLU + GCNConv + LayerNorm) on 8 TRN2 NeuronCores.

v2 design:
  - Nodes degree-sorted, dealt round-robin to 8 cores; 6250 dst nodes/core
    (padded to 6272 = 49 tiles of 128 lanes). Single storage tiling (no
    per-half re-tiling): tile/lane of a node is the same for gather targets
    and storage.
  - Global gather table layout is tile-group-major: 7 groups of [8,8,8,8,8,8,1]
    tiles; within a group rows are (core, tile, lane). Layer-1 table (dinv-
    scaled x, bf16) is built on host and passed as an input parameter; layer-2
    table is assembled with 7 chunked AllGathers issued as tile groups finish,
    overlapping the layer-1 gather stream.
  - Gather uses SIGNED int16 indices with the DMA base planted at table row
    32768: idx = row - 32768 spans [-32768, 17407], covering all 50176 rows in
    ONE stream (the Q7 ucode sign-extends idxs and IVP_MULUSAN multiplies them
    signed). This removes the H0/H1 split, the fold permutation matmuls, and
    all IS_EQ one-hot building. Only trailing-negative idxs are trimmed by the
    ucode, so each gather call must END on a non-negative idx — the planner
    reorders each call's last chunk to end on a pad (pads point at a
    guaranteed-zero dummy row, idx +17407).
  - Gather calls are spread round-robin over 4 SWDGE queues; each queue
    activates a different Q7 core pair, so descriptor generation for 4 calls
    proceeds in parallel.
  - Aggregation accumulates TRANSPOSED: matmul(lhsT=chunk, rhs=identity)
    gives psum[f, d], so the W matmul (lhsT=aggT, rhs=W) directly yields
    row-major conv[d, f'] — no per-tile PE transpose + copy.
  - b==0 in this problem, so the dst-side dinv scale is absorbed by
    LayerNorm's scale invariance; layer-1 outputs are re-scaled by dinv (and
    dummy lanes zeroed) when stored as next-layer table rows.
"""
import os
import numpy as np
import ml_dtypes

N = 50000
E = 600000
D = 128
NC = 8
P = 128
SHARD = 6272            # 49 * 128
TILES = 49
GROUP_SZ = [8, 8, 8, 8, 8, 8, 1]      # tiles per AllGather group
BASE_ROW = 32768        # gather base row (idx 0 lands here)
PAD_ROW = 50175         # (core 7, tile 48, lane 127) -> dummy zero row
LN_EPS = 1e-5
GB = 32                 # chunks (128 edges each) per dma_gather call
NQ = 4                  # SWDGE queues

bf16 = ml_dtypes.bfloat16


# ----------------------------------------------------------------------------
# Host-side planning (index-only preprocessing)
# ----------------------------------------------------------------------------

class Plan:
    pass


def _row_of(core, tile, lane):
    """Table row for (core, tile, lane): shard-major (matches AllGather concat)."""
    return core * SHARD + tile * P + lane


def build_plan(edge_index: np.ndarray) -> Plan:
    pl = Plan()
    src = edge_index[0].astype(np.int64)
    dst = edge_index[1].astype(np.int64)

    deg = np.bincount(dst, minlength=N) + 1          # incl. mandatory self-loop
    order = np.argsort(-deg, kind="stable")          # global degree desc
    core_of = np.empty(N, dtype=np.int64)
    core_of[order] = np.arange(N) % NC               # deal round-robin
    pos = np.empty(N, dtype=np.int64)
    for c in range(NC):
        shard = order[c::NC]                          # 6250 nodes, deg desc
        pos[shard] = np.arange(len(shard))
    tile_of = pos // P
    lane_of = pos % P
    row = _row_of(core_of, tile_of, lane_of)
    idx16 = row - BASE_ROW                           # signed, [-32768, 17407]

    node_at = np.full((NC, SHARD), -1, dtype=np.int64)
    for c in range(NC):
        shard = order[c::NC]
        node_at[c, pos[shard]] = shard
    pl.node_at = node_at
    pl.deg = deg
    pl.row = row

    deg_in = deg - 1
    # per-tile rounds, uniform over cores (SPMD identical programs)
    m = np.zeros(NC * TILES, dtype=np.int64)
    np.maximum.at(m, core_of * TILES + tile_of, deg_in)
    R_uni = m.reshape(NC, TILES).max(axis=0)
    assert R_uni.min() >= 1
    pl.R_uni = R_uni
    chunk_base = np.zeros(TILES + 1, dtype=np.int64)
    chunk_base[1:] = np.cumsum(R_uni)
    n_chunks = int(chunk_base[-1])
    pl.chunk_base = chunk_base
    pl.n_chunks = n_chunks
    tile_of_chunk = np.repeat(np.arange(TILES), R_uni)

    # round index for each edge: rank among edges with same dst
    eorder = np.argsort(dst, kind="stable")
    sd = dst[eorder]
    starts = np.r_[0, np.flatnonzero(sd[1:] != sd[:-1]) + 1]
    group_of = np.zeros(E, dtype=np.int64)
    group_of[starts[1:]] = 1
    group_of = np.cumsum(group_of)
    rounds_sorted = np.arange(E) - starts[group_of]
    rounds = np.empty(E, dtype=np.int64)
    rounds[eorder] = rounds_sorted

    # slot arrays per core: [n_chunks*128] of signed idx values (pad -> zero row)
    PAD_IDX = PAD_ROW - BASE_ROW
    slots = [np.full(n_chunks * P, PAD_IDX, dtype=np.int64) for _ in range(NC)]
    e_core = core_of[dst]
    e_slot = (chunk_base[tile_of[dst]] + rounds) * P + lane_of[dst]
    e_val = idx16[src]
    for c in range(NC):
        mm = e_core == c
        slots[c][e_slot[mm]] = e_val[mm]

    # calls: consecutive chunks, <= GB each. The ucode trims TRAILING negative
    # idxs, so each call's very last slot (lane 127 of its final chunk) must be
    # non-negative in EVERY core. Round order within a (tile, lane) is free per
    # core, so swap a pad (positive) or positive-edge round into that slot.
    sizes = []
    rem = n_chunks
    head = [16, 16]
    tail_budget = 96
    mid = rem - sum(head) - tail_budget
    for s in head:
        sizes.append(s)
    while mid > 0:
        s = min(GB, mid)
        sizes.append(s)
        mid -= s
    rem_tail = n_chunks - sum(sizes)
    while rem_tail > 0:
        s = min(16, rem_tail)
        sizes.append(s)
        rem_tail -= s
    starts = np.r_[0, np.cumsum(sizes)]
    calls = []                                       # list of lists of chunk ids
    for c0, c1_ in zip(starts[:-1], starts[1:]):
        chunks = list(range(int(c0), int(c1_)))
        final = None
        for cand in reversed(chunks):
            t = int(tile_of_chunk[cand])
            r = cand - int(chunk_base[t])
            rounds_sl = [(int(chunk_base[t]) + rr) * P + 127
                         for rr in range(int(R_uni[t]))]
            swaps = []                               # (core, slot_a, slot_b)
            ok = True
            for c in range(NC):
                sl = (int(chunk_base[t]) + r) * P + 127
                if slots[c][sl] >= 0:
                    continue                         # already safe
                cand_sl = [s for s in rounds_sl if slots[c][s] >= 0]
                if not cand_sl:
                    ok = False
                    break
                swaps.append((c, sl, cand_sl[-1]))
                cand_sl.pop()
            if ok:
                final = cand
                for c, a, bsl in swaps:
                    slots[c][a], slots[c][bsl] = slots[c][bsl], slots[c][a]
                break
        assert final is not None, f"no fixable final chunk in call at {c0}"
        chunks.remove(final)
        chunks.append(final)
        calls.append(chunks)
    pl.calls = calls
    pl.tile_of_chunk = tile_of_chunk

    def wrap(flat):                                  # [num] -> [128, num//16]
        num = len(flat)
        w = np.zeros((16, num // 16), dtype=np.int16)
        w[np.arange(num) % 16, np.arange(num) // 16] = flat.astype(np.int16)
        return np.tile(w, (8, 1))

    idx_in = []
    col_ranges = []
    for c in range(NC):
        parts = []
        col = 0
        for chunks in calls:
            seg = np.concatenate([slots[c][ch * P:(ch + 1) * P]
                                  for ch in chunks])
            parts.append(wrap(seg))
            if c == 0:
                col_ranges.append((col, col + len(seg) // 16))
            col += len(seg) // 16
        idx_in.append(np.concatenate(parts, axis=1))
    pl.idx_in = idx_in
    pl.col_ranges = col_ranges
    return pl


def host_inputs(pl, inputs):
    """Per-core input tensors (elementwise/reindex preprocessing only)."""
    x = np.asarray(inputs["x"], dtype=np.float32)
    deg = pl.deg
    dinv_n = 1.0 / np.sqrt(deg.astype(np.float64))

    # global layer-1 table: dinv-scaled x rows in table layout, bf16
    tab0 = np.zeros((50176, D), dtype=bf16)
    valid = pl.node_at >= 0
    for c in range(NC):
        nodes = pl.node_at[c][valid[c]]
        rows = pl.row[nodes]
        tab0[rows] = (x[nodes] * dinv_n[nodes][:, None]).astype(bf16)

    per_core = []
    for c in range(NC):
        nodes = pl.node_at[c]
        v = nodes >= 0
        pidx = np.arange(SHARD)
        # local rows [lane, tile, feat] (same values as tab0 own-shard rows)
        xloc = np.zeros((P, TILES, D), dtype=bf16)
        xloc[pidx[v] % P, pidx[v] // P] = (
            x[nodes[v]] * dinv_n[nodes[v]][:, None]).astype(bf16)
        dinv_t = np.ones((P, TILES), dtype=np.float32)
        dinv_t[pidx[v] % P, pidx[v] // P] = dinv_n[nodes[v]].astype(np.float32)
        dinvm = dinv_t[:, TILES - 1:TILES].copy()
        dinvm[pidx[~v] % P] = 0.0                     # zero dummy lanes (tile 48)
        ident_h = np.eye(P, dtype=bf16)
        wflat = np.full(P, PAD_ROW - BASE_ROW, dtype=np.int64)
        ww = np.zeros((16, P // 16), dtype=np.int16)
        ww[np.arange(P) % 16, np.arange(P) // 16] = wflat.astype(np.int16)
        m = {
            "tab0": tab0,
            "ident": ident_h,
            "widx": np.tile(ww, (8, 1)),
            "xloc": xloc.reshape(P, TILES * D),
            "dinv": dinv_t,
            "dinvm": dinvm,
            "idx": pl.idx_in[c],
            "W1": np.asarray(inputs["W1"], np.float32),
            "W2": np.asarray(inputs["W2"], np.float32),
        }
        for nm in ["b1", "g1", "beta1", "b2", "g2", "beta2"]:
            m[nm] = np.tile(np.asarray(inputs[nm], np.float32)[None, :], (P, 1))
        per_core.append(m)
    return per_core


# ----------------------------------------------------------------------------
# Numpy emulation of the device program (for validating the plan quickly)
# ----------------------------------------------------------------------------

def emulate2(pl, inputs):
    """Faithful emulation consuming the WRAPPED idx tensors exactly as the
    device would (validates slot packing, call reordering, signed idxs)."""
    W = [np.asarray(inputs["W1"], np.float32), np.asarray(inputs["W2"], np.float32)]
    b = [np.asarray(inputs["b1"], np.float32), np.asarray(inputs["b2"], np.float32)]
    g = [np.asarray(inputs["g1"], np.float32), np.asarray(inputs["g2"], np.float32)]
    be = [np.asarray(inputs["beta1"], np.float32), np.asarray(inputs["beta2"], np.float32)]
    per_core = host_inputs(pl, inputs)

    def tobf(a):
        return a.astype(bf16).astype(np.float32)

    tab = np.asarray(per_core[0]["tab0"]).astype(np.float32)   # layer-1 table
    h1g_all = [None] * NC
    out_full = np.zeros((N, D), dtype=np.float32)

    for layer in range(2):
        ntab = np.zeros((50176, D), dtype=np.float32)
        for c in range(NC):
            xs = np.asarray(per_core[c]["xloc"], np.float32).reshape(P, TILES, D)
            if layer == 1:
                xs = h1g_all[c]
            dinv_t = np.asarray(per_core[c]["dinv"], np.float32)
            dinvm = np.asarray(per_core[c]["dinvm"], np.float32)

            psT = {}                                   # tile -> [D, P] accum
            remaining = {t: int(pl.R_uni[t]) for t in range(TILES)}
            for ci, chunks in enumerate(pl.calls):
                c0, c1 = pl.col_ranges[ci]
                wrapped = pl.idx_in[c][:16, c0:c1].astype(np.int64)
                num = (c1 - c0) * 16
                flat = np.empty(num, dtype=np.int64)
                ar = np.arange(num)
                flat[ar] = wrapped[ar % 16, ar // 16]
                # emulate ucode trailing-negative trim
                nn = num
                while nn > 0 and flat[nn - 1] < 0:
                    nn -= 1
                assert nn == num, f"call {ci} would be trimmed! (core {c})"
                rows = flat + BASE_ROW
                gath = (tab if layer == 0 else ntab_prev)[rows].reshape(-1, P, D)
                for i, ch in enumerate(chunks):
                    t = int(pl.tile_of_chunk[ch])
                    if t not in psT:
                        psT[t] = xs[:, t, :].T.copy()   # self-loop opens tile
                    psT[t] += gath[i].T
                    remaining[t] -= 1
                    if remaining[t] == 0:
                        aggT = tobf(psT.pop(t))          # bf16 eviction
                        conv = aggT.T @ tobf(W[layer])   # [d, f']
                        cb = conv + b[layer][None, :]
                        mu = cb.mean(axis=1, keepdims=True)
                        ctr = cb - mu
                        var = (ctr ** 2).mean(axis=1, keepdims=True)
                        o = ctr / np.sqrt(var + LN_EPS) * g[layer][None, :] + be[layer][None, :]
                        if layer == 0:
                            o = np.maximum(o, 0.0)
                            dcol = dinvm[:, 0] if t == TILES - 1 else dinv_t[:, t]
                            h1g_all[c] = h1g_all[c] if h1g_all[c] is not None else \
                                np.zeros((P, TILES, D), dtype=np.float32)
                            h1g_all[c][:, t, :] = tobf(o * dcol[:, None])
                        else:
                            outs = o
                            pidx = np.arange(t * P, (t + 1) * P)
                            nodes = pl.node_at[c][pidx]
                            v = nodes >= 0
                            out_full[nodes[v]] = o[v]
            assert not psT, f"unclosed tiles {list(psT)} core {c}"
            if layer == 0:
                # core's h1g rows -> next-layer table (AllGather emulation)
                for t in range(TILES):
                    rows = _row_of(c, t, np.arange(P))
                    ntab[rows] = h1g_all[c][:, t, :]
        ntab_prev = ntab
    return out_full


# ----------------------------------------------------------------------------
# Bass kernel
# ----------------------------------------------------------------------------

def build_bass(pl, triv):
    import concourse.bacc as bacc
    import concourse.mybir as mybir
    import concourse.tile as tile

    f32 = mybir.dt.float32
    bf = mybir.dt.bfloat16
    AF = mybir.ActivationFunctionType
    OP = mybir.AluOpType

    nc = bacc.Bacc("TRN2", target_bir_lowering=False, debug=False, num_devices=NC,
                   num_swdge_queues=NQ, dynamic_dma_scratch_size=16384)

    tab0_ext = nc.declare_dram_parameter("tab0", [50176, D], bf, isOutput=False)
    xloc_ext = nc.declare_dram_parameter("xloc", [P, TILES * D], bf, isOutput=False)
    dinv_ext = nc.declare_dram_parameter("dinv", [P, TILES], f32, isOutput=False)
    dinvm_ext = nc.declare_dram_parameter("dinvm", [P, 1], f32, isOutput=False)
    totcols = pl.idx_in[0].shape[1]
    idx_ext = nc.declare_dram_parameter("idx", [P, totcols], mybir.dt.int16, isOutput=False)
    widx_ext = nc.declare_dram_parameter("widx", [P, P // 16], mybir.dt.int16, isOutput=False)
    ident_ext = nc.declare_dram_parameter("ident", [P, P], bf, isOutput=False)
    W_ext = [nc.declare_dram_parameter(f"W{i+1}", [D, D], f32, isOutput=False) for i in range(2)]
    vecs_ext = {}
    for nm in ["b1", "g1", "beta1", "b2", "g2", "beta2"]:
        vecs_ext[nm] = nc.declare_dram_parameter(nm, [P, D], f32, isOutput=False)
    out_ext = nc.declare_dram_parameter("out", [SHARD, D], f32, isOutput=True)

    # group tile ranges
    gstart = [0, 8, 16, 24, 32, 40, 48]
    grows_in = [(gs * P, (gs + sz) * P) for gs, sz in zip(gstart, GROUP_SZ)]
    grows_out = []
    rb = 0
    for sz in GROUP_SZ:
        grows_out.append((rb, rb + sz * P * NC))
        rb += sz * P * NC

    with tile.TileContext(nc) as tc:
        with tc.tile_pool(name="const", bufs=1) as cpool, \
             tc.tile_pool(name="store", bufs=1) as spool, \
             tc.tile_pool(name="g", bufs=10) as gpool, \
             tc.tile_pool(name="work", bufs=4) as wpool, \
             tc.tile_pool(name="psA", bufs=5, space="PSUM") as psA, \
             tc.tile_pool(name="psC", bufs=3, space="PSUM") as psC, \
             tc.tile_pool(name="dram", bufs=1, space="DRAM") as dpool:

            ident_bf = cpool.tile([P, P], bf)
            nc.sync.dma_start(out=ident_bf[:], in_=ident_ext[:])

            widx_t = cpool.tile([P, P // 16], mybir.dt.int16)
            nc.sync.dma_start(out=widx_t[:], in_=widx_ext[:])
            idx_t = cpool.tile([P, totcols], mybir.dt.int16)
            head_cols = pl.col_ranges[1][1]          # first two calls' columns
            nc.sync.dma_start(out=idx_t[:, :head_cols], in_=idx_ext[:, :head_cols])
            nc.sync.dma_start(out=idx_t[:, head_cols:], in_=idx_ext[:, head_cols:])

            xs_store = spool.tile([P, TILES, D], bf)
            nc.sync.dma_start(
                out=xs_store[:].rearrange("l t f -> l (t f)"), in_=xloc_ext[:])

            Wbf = []
            for i in range(2):
                wt = cpool.tile([P, D], f32, name=f"w32_{i}")
                nc.sync.dma_start(out=wt[:], in_=W_ext[i][:])
                wb = cpool.tile([P, D], bf, name=f"wbf_{i}")
                nc.vector.tensor_copy(out=wb[:], in_=wt[:])
                Wbf.append(wb)

            vecs = {}
            for nm in vecs_ext:
                vt = cpool.tile([P, D], f32, name=f"vec_{nm}")
                nc.sync.dma_start(out=vt[:], in_=vecs_ext[nm][:])
                vecs[nm] = vt

            dinv = cpool.tile([P, TILES], f32)
            nc.sync.dma_start(out=dinv[:], in_=dinv_ext[:])
            dinvm = cpool.tile([P, 1], f32)
            nc.sync.dma_start(out=dinvm[:], in_=dinvm_ext[:])
            dinvn = cpool.tile([P, TILES], f32)
            nc.vector.tensor_scalar(out=dinvn[:], in0=dinv[:], scalar1=-1.0,
                                    scalar2=None, op0=OP.mult)
            dinvmn = cpool.tile([P, 1], f32)
            nc.vector.tensor_scalar(out=dinvmn[:], in0=dinvm[:], scalar1=-1.0,
                                    scalar2=None, op0=OP.mult)
            one_t = cpool.tile([P, 1], f32)
            nc.vector.memset(one_t[:], 1.0)
            negone_t = cpool.tile([P, 1], f32)
            nc.vector.memset(negone_t[:], -1.0)
            eps_t = cpool.tile([P, 1], f32)
            nc.vector.memset(eps_t[:], float(LN_EPS))

            h1g_store = spool.tile([P, TILES, D], bf)

            cc_in = dpool.tile([SHARD, D], bf, name="ccin")
            table2 = dpool.tile([NC * SHARD, D], bf, name="table2",
                                addr_space="Shared")

            def finish_tile(layer, t, psT):
                b_triv, g_triv, be_triv = triv[layer]
                s_aggT = wpool.tile([P, D], bf, tag="saggT", name=f"saT_{layer}_{t}")
                nc.scalar.activation(out=s_aggT[:], in_=psT[:], func=AF.Identity)
                convp = psC.tile([P, D], f32, space="PSUM", tag="conv",
                                 name=f"conv_{layer}_{t}")
                nc.tensor.matmul(out=convp[:], lhsT=s_aggT[:], rhs=Wbf[layer][:],
                                 start=True, stop=True)

                if b_triv and g_triv and be_triv:
                    # LayerNorm via bn_stats/bn_aggr; dst-side dinv absorbed by
                    # LN scale invariance (layer 0 re-applies it for the next
                    # table; layer 1 emits LN directly).
                    bn6 = wpool.tile([P, 6], f32, tag="bn6", name=f"bn6_{layer}_{t}")
                    nc.vector.bn_stats(out=bn6[:], in_=convp[:])
                    mv = wpool.tile([P, 2], f32, tag="mv", name=f"mv_{layer}_{t}")
                    nc.vector.bn_aggr(out=mv[:], in_=bn6[:])
                    std = wpool.tile([P, 1], f32, tag="std", name=f"std_{layer}_{t}")
                    nc.scalar.activation(out=std[:], in_=mv[:, 1:2], func=AF.Sqrt,
                                         bias=eps_t[:, 0:1])
                    rstd = wpool.tile([P, 1], f32, tag="rstd", name=f"rstd_{layer}_{t}")
                    nc.vector.reciprocal(rstd[:], std[:])
                    if layer == 0:
                        dcol = dinvm[:, 0:1] if t == TILES - 1 else dinv[:, t:t + 1]
                        dcoln = dinvmn[:, 0:1] if t == TILES - 1 else dinvn[:, t:t + 1]
                    else:
                        dcol = one_t[:, 0:1]
                        dcoln = negone_t[:, 0:1]
                    rsd = wpool.tile([P, 1], f32, tag="rsd", name=f"rsd_{layer}_{t}")
                    nc.vector.tensor_scalar(out=rsd[:], in0=rstd[:], scalar1=dcol,
                                            scalar2=None, op0=OP.mult)
                    rsdn = wpool.tile([P, 1], f32, tag="rsdn", name=f"rsdn_{layer}_{t}")
                    nc.vector.tensor_scalar(out=rsdn[:], in0=rstd[:], scalar1=dcoln,
                                            scalar2=None, op0=OP.mult)
                    nmr = wpool.tile([P, 1], f32, tag="nmr", name=f"nmr_{layer}_{t}")
                    nc.vector.tensor_scalar(out=nmr[:], in0=mv[:, 0:1],
                                            scalar1=rsdn[:, 0:1], scalar2=None,
                                            op0=OP.mult)
                    if layer == 0:
                        nc.scalar.activation(out=h1g_store[:, t, :], in_=convp[:],
                                             func=AF.Relu, scale=rsd[:, 0:1],
                                             bias=nmr[:, 0:1])
                    else:
                        o1 = wpool.tile([P, D], f32, tag="o1", name=f"o1f_{t}")
                        nc.scalar.activation(out=o1[:], in_=convp[:],
                                             func=AF.Identity, scale=rsd[:, 0:1],
                                             bias=nmr[:, 0:1])
                        nc.sync.dma_start(out=out_ext[t * P:(t + 1) * P, :], in_=o1[:])
                    return

                # general path (b/g/beta nontrivial)
                if b_triv:
                    cb_ap = convp[:]
                else:
                    sc = wpool.tile([P, D], f32, tag="sc", name=f"sc_{layer}_{t}")
                    dcol = dinvm[:, 0:1] if t == TILES - 1 else dinv[:, t:t + 1]
                    nc.scalar.activation(out=sc[:], in_=convp[:], func=AF.Identity,
                                         scale=dcol)
                    bv = vecs["b1" if layer == 0 else "b2"]
                    cb = wpool.tile([P, D], f32, tag="cb", name=f"cb_{layer}_{t}")
                    nc.vector.tensor_tensor(out=cb[:], in0=sc[:], in1=bv[:], op=OP.add)
                    cb_ap = cb[:]
                scr = wpool.tile([P, D], f32, tag="scr", name=f"scr_{layer}_{t}")
                negmu = wpool.tile([P, 1], f32, tag="negmu", name=f"negmu_{layer}_{t}")
                nc.scalar.activation(out=scr[:], in_=cb_ap, func=AF.Identity,
                                     scale=-1.0 / D, accum_out=negmu[:])
                ctr = wpool.tile([P, D], f32, tag="ctr", name=f"ctr_{layer}_{t}")
                nc.scalar.activation(out=ctr[:], in_=cb_ap, func=AF.Identity,
                                     bias=negmu[:, 0:1])
                sqs = wpool.tile([P, D], f32, tag="sqs", name=f"sqs_{layer}_{t}")
                var_raw = wpool.tile([P, 1], f32, tag="varr", name=f"varr_{layer}_{t}")
                nc.scalar.activation(out=sqs[:], in_=ctr[:], func=AF.Square,
                                     scale=float(1.0 / np.sqrt(D)),
                                     accum_out=var_raw[:])
                std = wpool.tile([P, 1], f32, tag="std", name=f"std_{layer}_{t}")
                nc.scalar.activation(out=std[:], in_=var_raw[:], func=AF.Sqrt,
                                     bias=eps_t[:, 0:1])
                rstd = wpool.tile([P, 1], f32, tag="rstd", name=f"rstd_{layer}_{t}")
                nc.vector.reciprocal(rstd[:], std[:])
                gv = vecs["g1" if layer == 0 else "g2"]
                bev = vecs["beta1" if layer == 0 else "beta2"]
                o1 = wpool.tile([P, D], f32, tag="o1", name=f"o1_{layer}_{t}")
                nc.scalar.activation(out=o1[:], in_=ctr[:], func=AF.Identity,
                                     scale=rstd[:, 0:1])
                o2 = wpool.tile([P, D], f32, tag="o2", name=f"o2_{layer}_{t}")
                nc.vector.tensor_tensor(out=o2[:], in0=o1[:], in1=gv[:], op=OP.mult)
                o3 = wpool.tile([P, D], f32, tag="o3", name=f"o3_{layer}_{t}")
                nc.vector.tensor_tensor(out=o3[:], in0=o2[:], in1=bev[:], op=OP.add)
                if layer == 0:
                    o4 = wpool.tile([P, D], f32, tag="o4", name=f"o4_{t}")
                    nc.scalar.activation(out=o4[:], in_=o3[:], func=AF.Relu)
                    dcol = dinvm[:, 0:1] if t == TILES - 1 else dinv[:, t:t + 1]
                    nc.vector.tensor_scalar(out=h1g_store[:, t, :], in0=o4[:],
                                            scalar1=dcol, scalar2=None,
                                            op0=OP.mult)
                else:
                    nc.sync.dma_start(out=out_ext[t * P:(t + 1) * P, :], in_=o3[:])

            # --- layer runner with deferred AllGather firing ---
            def run_layer2(layer):
                selfstore = xs_store if layer == 0 else h1g_store
                table_ap = (tab0_ext if layer == 0 else table2)[BASE_ROW:, :]
                open_ps = {}
                remaining = {t: int(pl.R_uni[t]) for t in range(TILES)}
                tiles_done = 0
                next_group = 0
                ag_ready = []                           # (group, ready_at_call)
                ag_fired = []                           # (group, fired_at_call)

                def pump_ags(ci, flush=False):
                    # one whole-table AllGather at layer end (the collective
                    # instruction busy-waits on gpsimd, so chunking does not
                    # overlap anything and only multiplies barrier cost)
                    if flush and not pump_ags.done:
                        pump_ags.done = True
                        nc.gpsimd.collective_compute(
                            "AllGather", OP.bypass,
                            replica_groups=[list(range(NC))],
                            ins=[cc_in[:].opt()],
                            outs=[table2[:].opt()],
                        )
                pump_ags.done = False

                # warm-up: one tiny pad-only gather per queue absorbs the
                # per-layer stream serialization before real calls arrive
                for wr in range(2 * NQ):
                    wg = gpool.tile([P, 1, D], bf, tag="warm",
                                    name=f"warm_{layer}_{wr}")
                    nc.gpsimd.dma_gather(
                        out_ap=wg[:],
                        in_ap=table_ap,
                        idxs_ap=widx_t[:],
                        num_idxs=P,
                        num_idxs_reg=P,
                        elem_size=D,
                        single_packet=False,
                        queue_num=wr % NQ,
                    )
                for ci, chunks in enumerate(pl.calls):
                    c0, c1 = pl.col_ranges[ci]
                    nch = len(chunks)
                    gbuf = gpool.tile([P, GB, D], bf, tag="g",
                                      name=f"g_{layer}_{ci}")
                    nc.gpsimd.dma_gather(
                        out_ap=gbuf[:, :nch, :],
                        in_ap=table_ap,
                        idxs_ap=idx_t[:, c0:c1],
                        num_idxs=nch * P,
                        num_idxs_reg=nch * P,
                        elem_size=D,
                        single_packet=False,
                        queue_num=ci % NQ,
                    )
                    if layer == 0 and os.environ.get("V2_AG_INTERLEAVE", "1") == "1":
                        pump_ags(ci)
                    for i, ch in enumerate(chunks):
                        t = int(pl.tile_of_chunk[ch])
                        if t not in open_ps:
                            psT = psA.tile([P, D], f32, space="PSUM", tag="agg",
                                           name=f"ps_{layer}_{t}")
                            nc.tensor.matmul(out=psT[:], lhsT=selfstore[:, t, :],
                                             rhs=ident_bf[:], start=True, stop=False)
                            open_ps[t] = psT
                        psT = open_ps[t]
                        remaining[t] -= 1
                        last = remaining[t] == 0
                        nc.tensor.matmul(out=psT[:], lhsT=gbuf[:, i, :],
                                         rhs=ident_bf[:], start=False, stop=last)
                        if last:
                            finish_tile(layer, t, open_ps.pop(t))
                            tiles_done += 1
                            if layer == 0 and next_group < 7 and \
                                    tiles_done == gstart[next_group] + GROUP_SZ[next_group]:
                                r0, r1 = grows_in[next_group]
                                sz = GROUP_SZ[next_group]
                                nc.sync.dma_start(
                                    out=cc_in[r0:r1, :].rearrange(
                                        "(t l) f -> l t f", t=sz),
                                    in_=h1g_store[:, gstart[next_group]:
                                                  gstart[next_group] + sz, :])
                                ag_ready.append((next_group, ci))
                                next_group += 1
                assert not open_ps
                if layer == 0:
                    assert next_group == 7, next_group
                    pump_ags(len(pl.calls), flush=True)
                    pump_ags(len(pl.calls), flush=True)

            run_layer2(0)
            run_layer2(1)

    nc.compile()
    return nc


# ----------------------------------------------------------------------------
# Entry point
# ----------------------------------------------------------------------------

_last_result = None


def kernel(**inputs) -> np.ndarray:
    edge_index = np.asarray(inputs["edge_index"])
    pl = build_plan(edge_index)

    if os.environ.get("KERNEL_EMULATE") == "1":
        return emulate2(pl, inputs)

    from concourse.bass_utils import run_bass_kernel_spmd
    triv = []
    for i in (1, 2):
        triv.append((
            not np.any(np.asarray(inputs[f"b{i}"])),
            np.all(np.asarray(inputs[f"g{i}"]) == 1.0),
            not np.any(np.asarray(inputs[f"beta{i}"])),
        ))
    nc = build_bass(pl, triv)

    in_maps = host_inputs(pl, inputs)

    kw = {}
    if os.environ.get("KERNEL_TRACE") == "1":
        kw = dict(trace=True, trace_cores=[0])
    res = run_bass_kernel_spmd(nc, in_maps, core_ids=list(range(NC)), **kw)
    global _last_result
    _last_result = res

    out = np.zeros((N, D), dtype=np.float32)
    for c in range(NC):
        o = np.asarray(res.results[c]["out"], dtype=np.float32)
        valid = pl.node_at[c] >= 0
        out[pl.node_at[c][valid]] = o[valid]
    return out


# revision 32
# speedup vs baseline: 1.1055x; 1.1055x over previous
"""2-layer GCN (GCNConv + LayerNorm + ReLU + GCNConv + LayerNorm) on 8 TRN2 NeuronCores.

v2 design:
  - Nodes degree-sorted, dealt round-robin to 8 cores; 6250 dst nodes/core
    (padded to 6272 = 49 tiles of 128 lanes). Single storage tiling (no
    per-half re-tiling): tile/lane of a node is the same for gather targets
    and storage.
  - Global gather table layout is tile-group-major: 7 groups of [8,8,8,8,8,8,1]
    tiles; within a group rows are (core, tile, lane). Layer-1 table (dinv-
    scaled x, bf16) is built on host and passed as an input parameter; layer-2
    table is assembled with 7 chunked AllGathers issued as tile groups finish,
    overlapping the layer-1 gather stream.
  - Gather uses SIGNED int16 indices with the DMA base planted at table row
    32768: idx = row - 32768 spans [-32768, 17407], covering all 50176 rows in
    ONE stream (the Q7 ucode sign-extends idxs and IVP_MULUSAN multiplies them
    signed). This removes the H0/H1 split, the fold permutation matmuls, and
    all IS_EQ one-hot building. Only trailing-negative idxs are trimmed by the
    ucode, so each gather call must END on a non-negative idx — the planner
    reorders each call's last chunk to end on a pad (pads point at a
    guaranteed-zero dummy row, idx +17407).
  - Gather calls are spread round-robin over 4 SWDGE queues; each queue
    activates a different Q7 core pair, so descriptor generation for 4 calls
    proceeds in parallel.
  - Aggregation accumulates TRANSPOSED: matmul(lhsT=chunk, rhs=identity)
    gives psum[f, d], so the W matmul (lhsT=aggT, rhs=W) directly yields
    row-major conv[d, f'] — no per-tile PE transpose + copy.
  - b==0 in this problem, so the dst-side dinv scale is absorbed by
    LayerNorm's scale invariance; layer-1 outputs are re-scaled by dinv (and
    dummy lanes zeroed) when stored as next-layer table rows.
"""
import os
import numpy as np
import ml_dtypes

N = 50000
E = 600000
D = 128
NC = 8
P = 128
SHARD = 6272            # 49 * 128
TILES = 49
GROUP_SZ = [8, 8, 8, 8, 8, 8, 1]      # tiles per AllGather group
BASE_ROW = 32768        # gather base row (idx 0 lands here)
PAD_ROW = 50175         # (core 7, tile 48, lane 127) -> dummy zero row
LN_EPS = 1e-5
GB = 32                 # chunks (128 edges each) per dma_gather call
NQ = 4                  # SWDGE queues

bf16 = ml_dtypes.bfloat16


# ----------------------------------------------------------------------------
# Host-side planning (index-only preprocessing)
# ----------------------------------------------------------------------------

class Plan:
    pass


def _row_of(core, tile, lane):
    """Table row for (core, tile, lane): shard-major (matches AllGather concat)."""
    return core * SHARD + tile * P + lane


def build_plan(edge_index: np.ndarray) -> Plan:
    pl = Plan()
    src = edge_index[0].astype(np.int64)
    dst = edge_index[1].astype(np.int64)

    deg = np.bincount(dst, minlength=N) + 1          # incl. mandatory self-loop
    order = np.argsort(-deg, kind="stable")          # global degree desc
    core_of = np.empty(N, dtype=np.int64)
    core_of[order] = np.arange(N) % NC               # deal round-robin
    pos = np.empty(N, dtype=np.int64)
    for c in range(NC):
        shard = order[c::NC]                          # 6250 nodes, deg desc
        pos[shard] = np.arange(len(shard))
    tile_of = pos // P
    lane_of = pos % P
    row = _row_of(core_of, tile_of, lane_of)
    idx16 = row - BASE_ROW                           # signed, [-32768, 17407]

    node_at = np.full((NC, SHARD), -1, dtype=np.int64)
    for c in range(NC):
        shard = order[c::NC]
        node_at[c, pos[shard]] = shard
    pl.node_at = node_at
    pl.deg = deg
    pl.row = row

    deg_in = deg - 1
    # per-tile rounds, uniform over cores (SPMD identical programs)
    m = np.zeros(NC * TILES, dtype=np.int64)
    np.maximum.at(m, core_of * TILES + tile_of, deg_in)
    R_uni = m.reshape(NC, TILES).max(axis=0)
    assert R_uni.min() >= 1
    pl.R_uni = R_uni
    chunk_base = np.zeros(TILES + 1, dtype=np.int64)
    chunk_base[1:] = np.cumsum(R_uni)
    n_chunks = int(chunk_base[-1])
    pl.chunk_base = chunk_base
    pl.n_chunks = n_chunks
    tile_of_chunk = np.repeat(np.arange(TILES), R_uni)

    # round index for each edge: rank among edges with same dst
    eorder = np.argsort(dst, kind="stable")
    sd = dst[eorder]
    starts = np.r_[0, np.flatnonzero(sd[1:] != sd[:-1]) + 1]
    group_of = np.zeros(E, dtype=np.int64)
    group_of[starts[1:]] = 1
    group_of = np.cumsum(group_of)
    rounds_sorted = np.arange(E) - starts[group_of]
    rounds = np.empty(E, dtype=np.int64)
    rounds[eorder] = rounds_sorted

    # slot arrays per core: [n_chunks*128] of signed idx values (pad -> zero row)
    PAD_IDX = PAD_ROW - BASE_ROW
    slots = [np.full(n_chunks * P, PAD_IDX, dtype=np.int64) for _ in range(NC)]
    e_core = core_of[dst]
    e_slot = (chunk_base[tile_of[dst]] + rounds) * P + lane_of[dst]
    e_val = idx16[src]
    for c in range(NC):
        mm = e_core == c
        slots[c][e_slot[mm]] = e_val[mm]

    # calls: consecutive chunks, <= GB each. The ucode trims TRAILING negative
    # idxs, so each call's very last slot (lane 127 of its final chunk) must be
    # non-negative in EVERY core. Round order within a (tile, lane) is free per
    # core, so swap a pad (positive) or positive-edge round into that slot.
    sizes = []
    rem = n_chunks
    head = [16, 16]
    tail_budget = 96
    mid = rem - sum(head) - tail_budget
    for s in head:
        sizes.append(s)
    while mid > 0:
        s = min(GB, mid)
        sizes.append(s)
        mid -= s
    rem_tail = n_chunks - sum(sizes)
    while rem_tail > 0:
        s = min(16, rem_tail)
        sizes.append(s)
        rem_tail -= s
    starts = np.r_[0, np.cumsum(sizes)]
    calls = []                                       # list of lists of chunk ids
    for c0, c1_ in zip(starts[:-1], starts[1:]):
        chunks = list(range(int(c0), int(c1_)))
        final = None
        for cand in reversed(chunks):
            t = int(tile_of_chunk[cand])
            r = cand - int(chunk_base[t])
            rounds_sl = [(int(chunk_base[t]) + rr) * P + 127
                         for rr in range(int(R_uni[t]))]
            swaps = []                               # (core, slot_a, slot_b)
            ok = True
            for c in range(NC):
                sl = (int(chunk_base[t]) + r) * P + 127
                if slots[c][sl] >= 0:
                    continue                         # already safe
                cand_sl = [s for s in rounds_sl if slots[c][s] >= 0]
                if not cand_sl:
                    ok = False
                    break
                swaps.append((c, sl, cand_sl[-1]))
                cand_sl.pop()
            if ok:
                final = cand
                for c, a, bsl in swaps:
                    slots[c][a], slots[c][bsl] = slots[c][bsl], slots[c][a]
                break
        assert final is not None, f"no fixable final chunk in call at {c0}"
        chunks.remove(final)
        chunks.append(final)
        calls.append(chunks)
    pl.calls = calls
    pl.tile_of_chunk = tile_of_chunk

    def wrap(flat):                                  # [num] -> [128, num//16]
        num = len(flat)
        w = np.zeros((16, num // 16), dtype=np.int16)
        w[np.arange(num) % 16, np.arange(num) // 16] = flat.astype(np.int16)
        return np.tile(w, (8, 1))

    idx_in = []
    col_ranges = []
    for c in range(NC):
        parts = []
        col = 0
        for chunks in calls:
            seg = np.concatenate([slots[c][ch * P:(ch + 1) * P]
                                  for ch in chunks])
            parts.append(wrap(seg))
            if c == 0:
                col_ranges.append((col, col + len(seg) // 16))
            col += len(seg) // 16
        idx_in.append(np.concatenate(parts, axis=1))
    pl.idx_in = idx_in
    pl.col_ranges = col_ranges
    return pl


def host_inputs(pl, inputs):
    """Per-core input tensors (elementwise/reindex preprocessing only)."""
    x = np.asarray(inputs["x"], dtype=np.float32)
    deg = pl.deg
    dinv_n = 1.0 / np.sqrt(deg.astype(np.float64))

    # global layer-1 table: dinv-scaled x rows in table layout, bf16
    tab0 = np.zeros((50176, D), dtype=bf16)
    valid = pl.node_at >= 0
    for c in range(NC):
        nodes = pl.node_at[c][valid[c]]
        rows = pl.row[nodes]
        tab0[rows] = (x[nodes] * dinv_n[nodes][:, None]).astype(bf16)

    per_core = []
    for c in range(NC):
        nodes = pl.node_at[c]
        v = nodes >= 0
        pidx = np.arange(SHARD)
        # local rows [lane, tile, feat] (same values as tab0 own-shard rows)
        xloc = np.zeros((P, TILES, D), dtype=bf16)
        xloc[pidx[v] % P, pidx[v] // P] = (
            x[nodes[v]] * dinv_n[nodes[v]][:, None]).astype(bf16)
        dinv_t = np.ones((P, TILES), dtype=np.float32)
        dinv_t[pidx[v] % P, pidx[v] // P] = dinv_n[nodes[v]].astype(np.float32)
        dinvm = dinv_t[:, TILES - 1:TILES].copy()
        dinvm[pidx[~v] % P] = 0.0                     # zero dummy lanes (tile 48)
        ident_h = np.eye(P, dtype=bf16)
        wflat = np.full(P, PAD_ROW - BASE_ROW, dtype=np.int64)
        ww = np.zeros((16, P // 16), dtype=np.int16)
        ww[np.arange(P) % 16, np.arange(P) // 16] = wflat.astype(np.int16)
        m = {
            "tab0": tab0,
            "ident": ident_h,
            "widx": np.tile(ww, (8, 1)),
            "xloc": xloc.reshape(P, TILES * D),
            "dinv": dinv_t,
            "dinvm": dinvm,
            "idx": pl.idx_in[c],
            "W1": np.asarray(inputs["W1"], np.float32),
            "W2": np.asarray(inputs["W2"], np.float32),
        }
        for nm in ["b1", "g1", "beta1", "b2", "g2", "beta2"]:
            m[nm] = np.tile(np.asarray(inputs[nm], np.float32)[None, :], (P, 1))
        per_core.append(m)
    return per_core


# ----------------------------------------------------------------------------
# Numpy emulation of the device program (for validating the plan quickly)
# ----------------------------------------------------------------------------

def emulate2(pl, inputs):
    """Faithful emulation consuming the WRAPPED idx tensors exactly as the
    device would (validates slot packing, call reordering, signed idxs)."""
    W = [np.asarray(inputs["W1"], np.float32), np.asarray(inputs["W2"], np.float32)]
    b = [np.asarray(inputs["b1"], np.float32), np.asarray(inputs["b2"], np.float32)]
    g = [np.asarray(inputs["g1"], np.float32), np.asarray(inputs["g2"], np.float32)]
    be = [np.asarray(inputs["beta1"], np.float32), np.asarray(inputs["beta2"], np.float32)]
    per_core = host_inputs(pl, inputs)

    def tobf(a):
        return a.astype(bf16).astype(np.float32)

    tab = np.asarray(per_core[0]["tab0"]).astype(np.float32)   # layer-1 table
    h1g_all = [None] * NC
    out_full = np.zeros((N, D), dtype=np.float32)

    for layer in range(2):
        ntab = np.zeros((50176, D), dtype=np.float32)
        for c in range(NC):
            xs = np.asarray(per_core[c]["xloc"], np.float32).reshape(P, TILES, D)
            if layer == 1:
                xs = h1g_all[c]
            dinv_t = np.asarray(per_core[c]["dinv"], np.float32)
            dinvm = np.asarray(per_core[c]["dinvm"], np.float32)

            psT = {}                                   # tile -> [D, P] accum
            remaining = {t: int(pl.R_uni[t]) for t in range(TILES)}
            for ci, chunks in enumerate(pl.calls):
                c0, c1 = pl.col_ranges[ci]
                wrapped = pl.idx_in[c][:16, c0:c1].astype(np.int64)
                num = (c1 - c0) * 16
                flat = np.empty(num, dtype=np.int64)
                ar = np.arange(num)
                flat[ar] = wrapped[ar % 16, ar // 16]
                # emulate ucode trailing-negative trim
                nn = num
                while nn > 0 and flat[nn - 1] < 0:
                    nn -= 1
                assert nn == num, f"call {ci} would be trimmed! (core {c})"
                rows = flat + BASE_ROW
                gath = (tab if layer == 0 else ntab_prev)[rows].reshape(-1, P, D)
                for i, ch in enumerate(chunks):
                    t = int(pl.tile_of_chunk[ch])
                    if t not in psT:
                        psT[t] = xs[:, t, :].T.copy()   # self-loop opens tile
                    psT[t] += gath[i].T
                    remaining[t] -= 1
                    if remaining[t] == 0:
                        aggT = tobf(psT.pop(t))          # bf16 eviction
                        conv = aggT.T @ tobf(W[layer])   # [d, f']
                        cb = conv + b[layer][None, :]
                        mu = cb.mean(axis=1, keepdims=True)
                        ctr = cb - mu
                        var = (ctr ** 2).mean(axis=1, keepdims=True)
                        o = ctr / np.sqrt(var + LN_EPS) * g[layer][None, :] + be[layer][None, :]
                        if layer == 0:
                            o = np.maximum(o, 0.0)
                            dcol = dinvm[:, 0] if t == TILES - 1 else dinv_t[:, t]
                            h1g_all[c] = h1g_all[c] if h1g_all[c] is not None else \
                                np.zeros((P, TILES, D), dtype=np.float32)
                            h1g_all[c][:, t, :] = tobf(o * dcol[:, None])
                        else:
                            outs = o
                            pidx = np.arange(t * P, (t + 1) * P)
                            nodes = pl.node_at[c][pidx]
                            v = nodes >= 0
                            out_full[nodes[v]] = o[v]
            assert not psT, f"unclosed tiles {list(psT)} core {c}"
            if layer == 0:
                # core's h1g rows -> next-layer table (AllGather emulation)
                for t in range(TILES):
                    rows = _row_of(c, t, np.arange(P))
                    ntab[rows] = h1g_all[c][:, t, :]
        ntab_prev = ntab
    return out_full


# ----------------------------------------------------------------------------
# Bass kernel
# ----------------------------------------------------------------------------

def build_bass(pl, triv):
    import concourse.bacc as bacc
    import concourse.mybir as mybir
    import concourse.tile as tile

    f32 = mybir.dt.float32
    bf = mybir.dt.bfloat16
    AF = mybir.ActivationFunctionType
    OP = mybir.AluOpType

    nc = bacc.Bacc("TRN2", target_bir_lowering=False, debug=False, num_devices=NC,
                   num_swdge_queues=NQ, dynamic_dma_scratch_size=16384)

    tab0_ext = nc.declare_dram_parameter("tab0", [50176, D], bf, isOutput=False)
    xloc_ext = nc.declare_dram_parameter("xloc", [P, TILES * D], bf, isOutput=False)
    dinv_ext = nc.declare_dram_parameter("dinv", [P, TILES], f32, isOutput=False)
    dinvm_ext = nc.declare_dram_parameter("dinvm", [P, 1], f32, isOutput=False)
    totcols = pl.idx_in[0].shape[1]
    idx_ext = nc.declare_dram_parameter("idx", [P, totcols], mybir.dt.int16, isOutput=False)
    widx_ext = nc.declare_dram_parameter("widx", [P, P // 16], mybir.dt.int16, isOutput=False)
    ident_ext = nc.declare_dram_parameter("ident", [P, P], bf, isOutput=False)
    W_ext = [nc.declare_dram_parameter(f"W{i+1}", [D, D], f32, isOutput=False) for i in range(2)]
    vecs_ext = {}
    for nm in ["b1", "g1", "beta1", "b2", "g2", "beta2"]:
        vecs_ext[nm] = nc.declare_dram_parameter(nm, [P, D], f32, isOutput=False)
    out_ext = nc.declare_dram_parameter("out", [SHARD, D], f32, isOutput=True)

    # group tile ranges
    gstart = [0, 8, 16, 24, 32, 40, 48]
    grows_in = [(gs * P, (gs + sz) * P) for gs, sz in zip(gstart, GROUP_SZ)]
    grows_out = []
    rb = 0
    for sz in GROUP_SZ:
        grows_out.append((rb, rb + sz * P * NC))
        rb += sz * P * NC

    with tile.TileContext(nc) as tc:
        with tc.tile_pool(name="const", bufs=1) as cpool, \
             tc.tile_pool(name="store", bufs=1) as spool, \
             tc.tile_pool(name="g", bufs=10) as gpool, \
             tc.tile_pool(name="work", bufs=4) as wpool, \
             tc.tile_pool(name="psA", bufs=5, space="PSUM") as psA, \
             tc.tile_pool(name="psC", bufs=3, space="PSUM") as psC, \
             tc.tile_pool(name="dram", bufs=1, space="DRAM") as dpool:

            ident_bf = cpool.tile([P, P], bf)
            nc.sync.dma_start(out=ident_bf[:], in_=ident_ext[:])

            widx_t = cpool.tile([P, P // 16], mybir.dt.int16)
            nc.sync.dma_start(out=widx_t[:], in_=widx_ext[:])
            idx_t = cpool.tile([P, totcols], mybir.dt.int16)
            head_cols = pl.col_ranges[1][1]          # first two calls' columns
            nc.sync.dma_start(out=idx_t[:, :head_cols], in_=idx_ext[:, :head_cols])
            nc.sync.dma_start(out=idx_t[:, head_cols:], in_=idx_ext[:, head_cols:])

            xs_store = spool.tile([P, TILES, D], bf)
            nc.sync.dma_start(
                out=xs_store[:].rearrange("l t f -> l (t f)"), in_=xloc_ext[:])

            Wbf = []
            for i in range(2):
                wt = cpool.tile([P, D], f32, name=f"w32_{i}")
                nc.sync.dma_start(out=wt[:], in_=W_ext[i][:])
                wb = cpool.tile([P, D], bf, name=f"wbf_{i}")
                nc.vector.tensor_copy(out=wb[:], in_=wt[:])
                Wbf.append(wb)

            vecs = {}
            for nm in vecs_ext:
                vt = cpool.tile([P, D], f32, name=f"vec_{nm}")
                nc.sync.dma_start(out=vt[:], in_=vecs_ext[nm][:])
                vecs[nm] = vt

            dinv = cpool.tile([P, TILES], f32)
            nc.sync.dma_start(out=dinv[:], in_=dinv_ext[:])
            dinvm = cpool.tile([P, 1], f32)
            nc.sync.dma_start(out=dinvm[:], in_=dinvm_ext[:])
            dinvn = cpool.tile([P, TILES], f32)
            nc.vector.tensor_scalar(out=dinvn[:], in0=dinv[:], scalar1=-1.0,
                                    scalar2=None, op0=OP.mult)
            dinvmn = cpool.tile([P, 1], f32)
            nc.vector.tensor_scalar(out=dinvmn[:], in0=dinvm[:], scalar1=-1.0,
                                    scalar2=None, op0=OP.mult)
            one_t = cpool.tile([P, 1], f32)
            nc.vector.memset(one_t[:], 1.0)
            negone_t = cpool.tile([P, 1], f32)
            nc.vector.memset(negone_t[:], -1.0)
            eps_t = cpool.tile([P, 1], f32)
            nc.vector.memset(eps_t[:], float(LN_EPS))

            h1g_store = spool.tile([P, TILES, D], bf)

            cc_in = dpool.tile([SHARD, D], bf, name="ccin")
            table2 = dpool.tile([NC * SHARD, D], bf, name="table2",
                                addr_space="Shared")

            def finish_tile(layer, t, psT):
                b_triv, g_triv, be_triv = triv[layer]
                s_aggT = wpool.tile([P, D], bf, tag="saggT", name=f"saT_{layer}_{t}")
                nc.scalar.activation(out=s_aggT[:], in_=psT[:], func=AF.Identity)
                convp = psC.tile([P, D], f32, space="PSUM", tag="conv",
                                 name=f"conv_{layer}_{t}")
                nc.tensor.matmul(out=convp[:], lhsT=s_aggT[:], rhs=Wbf[layer][:],
                                 start=True, stop=True)

                if b_triv and g_triv and be_triv:
                    # LayerNorm via bn_stats/bn_aggr; dst-side dinv absorbed by
                    # LN scale invariance (layer 0 re-applies it for the next
                    # table; layer 1 emits LN directly).
                    bn6 = wpool.tile([P, 6], f32, tag="bn6", name=f"bn6_{layer}_{t}")
                    nc.vector.bn_stats(out=bn6[:], in_=convp[:])
                    mv = wpool.tile([P, 2], f32, tag="mv", name=f"mv_{layer}_{t}")
                    nc.vector.bn_aggr(out=mv[:], in_=bn6[:])
                    std = wpool.tile([P, 1], f32, tag="std", name=f"std_{layer}_{t}")
                    nc.scalar.activation(out=std[:], in_=mv[:, 1:2], func=AF.Sqrt,
                                         bias=eps_t[:, 0:1])
                    rstd = wpool.tile([P, 1], f32, tag="rstd", name=f"rstd_{layer}_{t}")
                    nc.vector.reciprocal(rstd[:], std[:])
                    if layer == 0:
                        dcol = dinvm[:, 0:1] if t == TILES - 1 else dinv[:, t:t + 1]
                        dcoln = dinvmn[:, 0:1] if t == TILES - 1 else dinvn[:, t:t + 1]
                    else:
                        dcol = one_t[:, 0:1]
                        dcoln = negone_t[:, 0:1]
                    rsd = wpool.tile([P, 1], f32, tag="rsd", name=f"rsd_{layer}_{t}")
                    nc.vector.tensor_scalar(out=rsd[:], in0=rstd[:], scalar1=dcol,
                                            scalar2=None, op0=OP.mult)
                    rsdn = wpool.tile([P, 1], f32, tag="rsdn", name=f"rsdn_{layer}_{t}")
                    nc.vector.tensor_scalar(out=rsdn[:], in0=rstd[:], scalar1=dcoln,
                                            scalar2=None, op0=OP.mult)
                    nmr = wpool.tile([P, 1], f32, tag="nmr", name=f"nmr_{layer}_{t}")
                    nc.vector.tensor_scalar(out=nmr[:], in0=mv[:, 0:1],
                                            scalar1=rsdn[:, 0:1], scalar2=None,
                                            op0=OP.mult)
                    if layer == 0:
                        nc.scalar.activation(out=h1g_store[:, t, :], in_=convp[:],
                                             func=AF.Relu, scale=rsd[:, 0:1],
                                             bias=nmr[:, 0:1])
                    else:
                        o1 = wpool.tile([P, D], f32, tag="o1", name=f"o1f_{t}")
                        nc.scalar.activation(out=o1[:], in_=convp[:],
                                             func=AF.Identity, scale=rsd[:, 0:1],
                                             bias=nmr[:, 0:1])
                        nc.sync.dma_start(out=out_ext[t * P:(t + 1) * P, :], in_=o1[:])
                    return

                # general path (b/g/beta nontrivial)
                if b_triv:
                    cb_ap = convp[:]
                else:
                    sc = wpool.tile([P, D], f32, tag="sc", name=f"sc_{layer}_{t}")
                    dcol = dinvm[:, 0:1] if t == TILES - 1 else dinv[:, t:t + 1]
                    nc.scalar.activation(out=sc[:], in_=convp[:], func=AF.Identity,
                                         scale=dcol)
                    bv = vecs["b1" if layer == 0 else "b2"]
                    cb = wpool.tile([P, D], f32, tag="cb", name=f"cb_{layer}_{t}")
                    nc.vector.tensor_tensor(out=cb[:], in0=sc[:], in1=bv[:], op=OP.add)
                    cb_ap = cb[:]
                scr = wpool.tile([P, D], f32, tag="scr", name=f"scr_{layer}_{t}")
                negmu = wpool.tile([P, 1], f32, tag="negmu", name=f"negmu_{layer}_{t}")
                nc.scalar.activation(out=scr[:], in_=cb_ap, func=AF.Identity,
                                     scale=-1.0 / D, accum_out=negmu[:])
                ctr = wpool.tile([P, D], f32, tag="ctr", name=f"ctr_{layer}_{t}")
                nc.scalar.activation(out=ctr[:], in_=cb_ap, func=AF.Identity,
                                     bias=negmu[:, 0:1])
                sqs = wpool.tile([P, D], f32, tag="sqs", name=f"sqs_{layer}_{t}")
                var_raw = wpool.tile([P, 1], f32, tag="varr", name=f"varr_{layer}_{t}")
                nc.scalar.activation(out=sqs[:], in_=ctr[:], func=AF.Square,
                                     scale=float(1.0 / np.sqrt(D)),
                                     accum_out=var_raw[:])
                std = wpool.tile([P, 1], f32, tag="std", name=f"std_{layer}_{t}")
                nc.scalar.activation(out=std[:], in_=var_raw[:], func=AF.Sqrt,
                                     bias=eps_t[:, 0:1])
                rstd = wpool.tile([P, 1], f32, tag="rstd", name=f"rstd_{layer}_{t}")
                nc.vector.reciprocal(rstd[:], std[:])
                gv = vecs["g1" if layer == 0 else "g2"]
                bev = vecs["beta1" if layer == 0 else "beta2"]
                o1 = wpool.tile([P, D], f32, tag="o1", name=f"o1_{layer}_{t}")
                nc.scalar.activation(out=o1[:], in_=ctr[:], func=AF.Identity,
                                     scale=rstd[:, 0:1])
                o2 = wpool.tile([P, D], f32, tag="o2", name=f"o2_{layer}_{t}")
                nc.vector.tensor_tensor(out=o2[:], in0=o1[:], in1=gv[:], op=OP.mult)
                o3 = wpool.tile([P, D], f32, tag="o3", name=f"o3_{layer}_{t}")
                nc.vector.tensor_tensor(out=o3[:], in0=o2[:], in1=bev[:], op=OP.add)
                if layer == 0:
                    o4 = wpool.tile([P, D], f32, tag="o4", name=f"o4_{t}")
                    nc.scalar.activation(out=o4[:], in_=o3[:], func=AF.Relu)
                    dcol = dinvm[:, 0:1] if t == TILES - 1 else dinv[:, t:t + 1]
                    nc.vector.tensor_scalar(out=h1g_store[:, t, :], in0=o4[:],
                                            scalar1=dcol, scalar2=None,
                                            op0=OP.mult)
                else:
                    nc.sync.dma_start(out=out_ext[t * P:(t + 1) * P, :], in_=o3[:])

            # --- layer runner with deferred AllGather firing ---
            def run_layer2(layer):
                selfstore = xs_store if layer == 0 else h1g_store
                table_ap = (tab0_ext if layer == 0 else table2)[BASE_ROW:, :]
                open_ps = {}
                remaining = {t: int(pl.R_uni[t]) for t in range(TILES)}
                tiles_done = 0
                next_group = 0
                ag_ready = []                           # (group, ready_at_call)
                ag_fired = []                           # (group, fired_at_call)

                def pump_ags(ci, flush=False):
                    # one whole-table AllGather at layer end (the collective
                    # instruction busy-waits on gpsimd, so chunking does not
                    # overlap anything and only multiplies barrier cost)
                    if flush and not pump_ags.done:
                        pump_ags.done = True
                        nc.gpsimd.collective_compute(
                            "AllGather", OP.bypass,
                            replica_groups=[list(range(NC))],
                            ins=[cc_in[:].opt()],
                            outs=[table2[:].opt()],
                        )
                pump_ags.done = False

                # warm-up: one tiny pad-only gather per queue absorbs the
                # per-layer stream serialization before real calls arrive
                for wq in range(NQ):
                    wg = gpool.tile([P, 1, D], bf, tag="warm",
                                    name=f"warm_{layer}_{wq}")
                    nc.gpsimd.dma_gather(
                        out_ap=wg[:],
                        in_ap=table_ap,
                        idxs_ap=widx_t[:],
                        num_idxs=P,
                        num_idxs_reg=P,
                        elem_size=D,
                        single_packet=False,
                        queue_num=wq,
                    )
                for ci, chunks in enumerate(pl.calls):
                    c0, c1 = pl.col_ranges[ci]
                    nch = len(chunks)
                    gbuf = gpool.tile([P, GB, D], bf, tag="g",
                                      name=f"g_{layer}_{ci}")
                    nc.gpsimd.dma_gather(
                        out_ap=gbuf[:, :nch, :],
                        in_ap=table_ap,
                        idxs_ap=idx_t[:, c0:c1],
                        num_idxs=nch * P,
                        num_idxs_reg=nch * P,
                        elem_size=D,
                        single_packet=False,
                        queue_num=ci % NQ,
                    )
                    if layer == 0 and os.environ.get("V2_AG_INTERLEAVE", "1") == "1":
                        pump_ags(ci)
                    for i, ch in enumerate(chunks):
                        t = int(pl.tile_of_chunk[ch])
                        if t not in open_ps:
                            psT = psA.tile([P, D], f32, space="PSUM", tag="agg",
                                           name=f"ps_{layer}_{t}")
                            nc.tensor.matmul(out=psT[:], lhsT=selfstore[:, t, :],
                                             rhs=ident_bf[:], start=True, stop=False)
                            open_ps[t] = psT
                        psT = open_ps[t]
                        remaining[t] -= 1
                        last = remaining[t] == 0
                        nc.tensor.matmul(out=psT[:], lhsT=gbuf[:, i, :],
                                         rhs=ident_bf[:], start=False, stop=last)
                        if last:
                            finish_tile(layer, t, open_ps.pop(t))
                            tiles_done += 1
                            if layer == 0 and next_group < 7 and \
                                    tiles_done == gstart[next_group] + GROUP_SZ[next_group]:
                                r0, r1 = grows_in[next_group]
                                sz = GROUP_SZ[next_group]
                                nc.sync.dma_start(
                                    out=cc_in[r0:r1, :].rearrange(
                                        "(t l) f -> l t f", t=sz),
                                    in_=h1g_store[:, gstart[next_group]:
                                                  gstart[next_group] + sz, :])
                                ag_ready.append((next_group, ci))
                                next_group += 1
                assert not open_ps
                if layer == 0:
                    assert next_group == 7, next_group
                    pump_ags(len(pl.calls), flush=True)
                    pump_ags(len(pl.calls), flush=True)

            run_layer2(0)
            run_layer2(1)

    nc.compile()
    return nc


# ----------------------------------------------------------------------------
# Entry point
# ----------------------------------------------------------------------------

_last_result = None


def kernel(**inputs) -> np.ndarray:
    edge_index = np.asarray(inputs["edge_index"])
    pl = build_plan(edge_index)

    if os.environ.get("KERNEL_EMULATE") == "1":
        return emulate2(pl, inputs)

    from concourse.bass_utils import run_bass_kernel_spmd
    triv = []
    for i in (1, 2):
        triv.append((
            not np.any(np.asarray(inputs[f"b{i}"])),
            np.all(np.asarray(inputs[f"g{i}"]) == 1.0),
            not np.any(np.asarray(inputs[f"beta{i}"])),
        ))
    nc = build_bass(pl, triv)

    in_maps = host_inputs(pl, inputs)

    kw = {}
    if os.environ.get("KERNEL_TRACE") == "1":
        kw = dict(trace=True, trace_cores=[0])
    res = run_bass_kernel_spmd(nc, in_maps, core_ids=list(range(NC)), **kw)
    global _last_result
    _last_result = res

    out = np.zeros((N, D), dtype=np.float32)
    for c in range(NC):
        o = np.asarray(res.results[c]["out"], dtype=np.float32)
        valid = pl.node_at[c] >= 0
        out[pl.node_at[c][valid]] = o[valid]
    return out
